# revision 12
# baseline (speedup 1.0000x reference)
"""Trainium2 Bass kernel for nn_Covariance_Metric (5-way 5-shot covariance metric).

Math (see reference):
  cov_w = centered-support covariance (512x512) per way w  [5 ways, 2205 samples each]
  sim[q,w,i] = q_i^T cov_w q_i   for each of 441 spatial positions i of query q
  scores[q,w] = conv_w . leaky_relu(sim[q,w,:]) + conv_b

Strategy: data-parallel over Q across 8 cores (19 queries/core, zero-padded).
Each core computes all 5 covariances (replicated) then its query shard.

Design:
  - Query positions are batched globally: 19*441 = 8379 positions, processed as
    66 chunks of 128 (partition dim = position), so every matmul runs with a
    full 128-row stationary operand (no 441 = 3*128+57 tail waste).
  - QMM_MODE="fp8res": query matmuls run fp8e4 DoubleRow (256-deep contraction,
    0.5 cyc/row) on q8 = fp8(q) AND the requantized residual r8 = fp8(q - q8),
    accumulated in the same PSUM group -> bf16-level accuracy at ~2x bf16 speed.
    The centered query master stays bf16 for the PE transposes that feed the
    DVE diag-product; fp8 tiles are produced per-chunk on the idle GpSimd.
  - Gram (covariance) phase: x2 is converted to fp8 on GpSimd, PE-transposed
    (1 cyc/row), and accumulated with fp8 DR matmuls; channel sums come from a
    ones-vector DR matmul on PE; mean correction is a rank-1 fp32r matmul.
    cov is stored as COV_SCALE*cov in fp8 (off-diagonals ~0.02 would fall into
    e4m3 denormals otherwise); LeakyReLU is positively homogeneous so the scale
    passes through to the scores and is undone on host in postprocess().
  - sim diag-product via scalar_tensor_tensor free-dim accumulate; most run on
    DVE straight from PSUM, every POOL_STT_EVERY-th way-chunk is bounced
    PSUM->SBUF-bf16 by the Scalar engine and reduced on GpSimd to balance load.
  - scores: LeakyReLU on DVE, then 66 accumulating mask-matmuls with conv_w
    scattered into a [128 pos, 66 chunk, 19 query] weight tensor; conv_b is
    added on host.
"""
import json
import numpy as np
from contextlib import ExitStack

import concourse.bass as bass
import concourse.tile as tile
from concourse import mybir
from concourse.masks import make_identity

# ---------------------------------------------------------------------------
# Workaround for this container's walrus build: it supports only ONE sync-wait
# command per instruction, while Tile attaches several. Rewrite the BIR JSON to
# hoist extra waits onto inserted same-engine NoOps (the NX sequencer processes
# them in order, so the gating is equivalent).
# ---------------------------------------------------------------------------

def _split_sync_waits_json(m: dict) -> int:
    nsplit = 0
    for fn in m["functions"]:
        for bb in fn["blocks"]:
            out = []
            for ins in bb["instructions"]:
                si = ins.get("sync_info")
                if si:
                    w = si.get("on_wait") or []
                    if len(w) > 1:
                        for c in w[:-1]:
                            nsplit += 1
                            out.append({
                                "debug": ins.get("debug", 0),
                                "engine": ins["engine"],
                                "ins": [], "outs": [],
                                "name": f"{ins['name']}-ws{nsplit}",
                                "opcode": "NoOp",
                                "sync_info": {"on_wait": [c], "on_update": []},
                            })
                        si["on_wait"] = [w[-1]]
                out.append(ins)
            bb["instructions"] = out
    return nsplit


_fixups_installed = False


def _install_fixups():
    global _fixups_installed
    if _fixups_installed:
        return
    _fixups_installed = True
    import concourse.bass_utils as bu
    import concourse.bass2jax as b2j

    orig = bu.compile_bir_kernel

    def compile_bir_kernel_patched(bir_json, tmpdir, neff_name="file.neff"):
        m = json.loads(bir_json)
        _split_sync_waits_json(m)
        return orig(json.dumps(m).encode(), tmpdir, neff_name)

    bu.compile_bir_kernel = compile_bir_kernel_patched
    b2j.compile_bir_kernel = compile_bir_kernel_patched


# ---------------------------------------------------------------------------
# Problem constants (hardcoded per contract)
# ---------------------------------------------------------------------------
Q, C, HW = 150, 512, 441
WAY, SHOT = 5, 5
NCORES = 8
QP = 19             # queries per core (8*19 = 152 >= 150, zero-padded)
CC = C // 128       # 4 c-chunks
NS = SHOT * HW      # 2205 samples per way
P = QP * HW         # 8379 positions per core
NCH = (P + 127) // 128   # 66 position chunks
PCH = NCH * 128          # 8448 padded positions
HWP = 512                # per-shot hw dim padded to 4 full 128-chunks
F32 = mybir.dt.float32
F32R = mybir.dt.float32r
BF16 = mybir.dt.bfloat16
FP8 = mybir.dt.float8e4
DR = mybir.MatmulPerfMode.DoubleRow

# tunables
QMM_MODE = "bf16"     # "bf16" | "fp8" | "fp8res" | "fp8x"
                      # fp8x: bf16 master for transposes/qT (exact diag
                      # multiplicand), per-chunk GpSimd fp8 copy for the
                      # DoubleRow matmuls (halves PE query time; cov fp8)
GRAM_FP8 = True       # gram matmuls in fp8 DoubleRow (else fp32r, no x2 pad)
POOL_STT_EVERY = 0    # 0 = all sim-reductions on DVE (TensorScalarPtr is
                      # not a legal GpSimd opcode on hw); k>0 kept for expts
                      # way-chunk bounced via ACT copy + GpSimd reduce
NBOUNCE = 0           # ways per chunk whose sim-reduction is bounced
                      # PSUM->SBUF-bf16 by Act, then STT on DVE in fast mode
                      # (measured: regression, Act relay stalls the pipeline)
PROD_FP8 = False      # write the discarded STT product in fp8 (halves the
                      # DVE SBUF write footprint per sim-reduction)
COV_SCALE = 64.0      # see module docstring

_cache = {}
HOST_BIAS = True      # conv_b (and 1/COV_SCALE) applied on host in kernel()


def postprocess(scores_dev, conv_b):
    return scores_dev * np.float32(1.0 / COV_SCALE) + np.float32(conv_b[0])


def make_in_map(x1p, x2, cw, cb, c):
    return {
        "x1s": np.ascontiguousarray(x1p[c * QP : (c + 1) * QP]),
        "x2": x2, "cw": cw,
    }


def _wq_segments():
    """(chunk t, dst partition range [a,b), query q, conv_w offset i0)."""
    segs = []
    for t in range(NCH):
        g0, g1 = 128 * t, 128 * t + 128
        q_lo, q_hi = g0 // HW, min((g1 - 1) // HW, QP - 1)
        for q in range(q_lo, q_hi + 1):
            s0, s1 = max(g0, q * HW), min(g1, (q + 1) * HW, P)
            if s1 > s0:
                segs.append((t, s0 - g0, s1 - g0, q, s0 - q * HW))
    return segs


def _build(repeat=1):
    nc = bass.Bass(trn_type="TRN2")
    x1s = nc.dram_tensor("x1s", [QP, C, HW], F32, kind="ExternalInput")
    x2 = nc.dram_tensor("x2", [WAY * SHOT, C, HW], F32, kind="ExternalInput")
    cw = nc.dram_tensor("cw", [HW], F32, kind="ExternalInput")
    scores = nc.dram_tensor("scores", [QP, WAY], F32, kind="ExternalOutput")

    AL = mybir.AluOpType
    AF = mybir.ActivationFunctionType
    QMDT = FP8 if QMM_MODE == "fp8" else BF16     # dtype of the query master
    GDT = FP8 if GRAM_FP8 else F32R

    with tile.TileContext(nc) as tc, ExitStack() as ctx:
        consts = ctx.enter_context(tc.tile_pool(name="consts", bufs=1))
        tr_ps = ctx.enter_context(tc.tile_pool(name="tr_ps", bufs=3, space="PSUM"))

        identq = consts.tile([128, 128], QMDT)
        make_identity(nc, identq[:])
        identg = consts.tile([128, 128], FP8 if GRAM_FP8 else F32)
        make_identity(nc, identg[:])
        # k-pair dim stride must be a multiple of 16 bytes for DoubleRow LDW
        ones_g = consts.tile([128, 2, 16], GDT)
        nc.gpsimd.memset(ones_g[:], 1.0)
        # conv_w scattered by (position-in-chunk, chunk, query):
        # Wq[p, t, q] = cw[128t + p - 441q] if 0 <= idx < 441 else 0.
        # Build a zero-padded DRAM copy cwpad[k] = cw[k-128] so each query is
        # ONE affine 2D DMA (the pad supplies the zero corners of the band).
        Wq = consts.tile([128, NCH + 1, QP], F32)
        nc.gpsimd.memset(Wq[:], 0.0)
        cwpad = nc.dram_tensor("cwpad", [768], F32, kind="Internal")
        zrow = consts.tile([1, 256], F32)
        nc.vector.memset(zrow[:], 0.0)
        nc.sync.dma_start(cwpad[0:128][None, :], zrow[0:1, 0:128])
        nc.sync.dma_start(cwpad[569:768][None, :], zrow[0:1, 0:199])
        nc.sync.dma_start(cwpad[128:569][None, :], cw[:][None, :])

        for _rep in range(repeat):
            with tc.tile_pool(name="rep", bufs=1) as repp:
                covdt = FP8 if QMM_MODE != "bf16" else BF16
                cov_w = [repp.tile([128, CC, C], covdt, name=f"cov{w}")
                         for w in range(WAY)]
                # centered queries, c-major, split at chunk boundaries so
                # early chunks only wait for their own queries' centering
                # (tile deps are whole-tile, not range-based)
                CH = [0, 17, 34, 50, 66]
                PS = [128 * c for c in CH]
                qns = [repp.tile([128, CC, PS[i + 1] - PS[i]], QMDT, name=f"qn{i}")
                       for i in range(4)]
                S_all = repp.tile([128, NCH, WAY], F32)
                nc.gpsimd.memset(qns[3][:, :, P - PS[3] :], 0.0)
                if QMM_MODE == "fp8d":
                    q8ns = [repp.tile([128, CC, PS[i + 1] - PS[i]], FP8,
                                      name=f"q8n{i}") for i in range(4)]
                    nc.gpsimd.memset(q8ns[3][:, :, P - PS[3] :], 0.0)

                def qn_chunk(t):
                    i = max(k for k in range(4) if CH[k] <= t)
                    return qns[i], slice(128 * t - PS[i], 128 * t - PS[i] + 128)

                # ---------------- covariance phase ----------------
                with tc.tile_pool(name="x2s", bufs=2) as x2pool, \
                     tc.tile_pool(name="x2f", bufs=2) as x2fp, \
                     tc.tile_pool(name="fT", bufs=3) as fTp, \
                     tc.tile_pool(name="murow", bufs=2) as murp, \
                     tc.tile_pool(name="g_ps", bufs=4, space="PSUM") as g_ps, \
                     tc.tile_pool(name="mu_ps", bufs=1, space="PSUM") as mu_ps:
                    for w in range(WAY):
                        g = [g_ps.tile([128, C], F32, name=f"g{w}_{j}", tag="g")
                             for j in range(CC)]
                        mrow_ps = mu_ps.tile([1, C], F32)
                        first = True
                        for s in range(SHOT):
                            x2s = x2pool.tile([128, CC, HW], F32)
                            nc.sync.dma_start(
                                x2s[:],
                                x2[w * SHOT + s].rearrange("(cc p) hw -> p cc hw", p=128),
                            )
                            # fp8 copy with zero-padded hw 441->512 (GpSimd)
                            x2f = x2fp.tile([128, CC, HWP], GDT)
                            nc.gpsimd.memset(x2f[:, :, HW:HWP], 0.0)
                            nc.gpsimd.tensor_copy(x2f[:, :, :HW], x2s[:])
                            for pair in range(2):   # hw chunks (2p, 2p+1)
                                fT = fTp.tile([128, 2, C], GDT)
                                for slot in range(2):
                                    h = 2 * pair + slot
                                    if GRAM_FP8:
                                        tpt = tr_ps.tile([128, C, 2], GDT, tag="tp")
                                        tp = tpt[:, :, 0]   # fp8 transpose needs
                                        # output element step of 2 (walrus)
                                    else:
                                        tpt = tr_ps.tile([128, C], GDT, tag="tp")
                                        tp = tpt[:, :]
                                    for j in range(CC):
                                        nc.tensor.transpose(
                                            tp[:, j * 128 : (j + 1) * 128],
                                            x2f[:, j, h * 128 : (h + 1) * 128],
                                            identg[:],
                                        )
                                    if (pair + slot) % 2 == 0:
                                        nc.scalar.copy(fT[:, slot, :], tp[:, :])
                                    else:
                                        nc.vector.tensor_copy(fT[:, slot, :], tp[:, :])
                                for j in range(CC):
                                    nc.tensor.matmul(
                                        g[j][:, :],
                                        lhsT=fT[:, :, j * 128 : (j + 1) * 128],
                                        rhs=fT[:, :, :],
                                        start=first, stop=False,
                                        perf_mode=DR,
                                    )
                                nc.tensor.matmul(
                                    mrow_ps[0:1, :],
                                    lhsT=ones_g[:, :, 0:1],
                                    rhs=fT[:, :, :],
                                    start=first, stop=(s == SHOT - 1 and pair == 1),
                                    perf_mode=DR,
                                )
                                first = False
                        murow = murp.tile([1, C], F32R)    # mu = sums / NS
                        nc.scalar.activation(murow[:], mrow_ps[:], AF.Copy, scale=1.0 / NS)
                        mursc = murp.tile([1, C], F32R)    # -NS*mu = -sums
                        nc.scalar.activation(mursc[:], mrow_ps[:], AF.Copy, scale=-1.0)
                        for j in range(CC):
                            nc.tensor.matmul(
                                g[j][:, :],
                                lhsT=mursc[0:1, j * 128 : (j + 1) * 128],
                                rhs=murow[0:1, :],
                                start=False, stop=True,
                            )
                        for j in range(CC):
                            nc.scalar.activation(
                                cov_w[w][:, j, :], g[j][:, :], AF.Copy,
                                scale=COV_SCALE / (NS - 1),
                            )

                # ---------------- query load + centering ----------------
                with tc.tile_pool(name="qraw", bufs=2) as qrawp, \
                     tc.tile_pool(name="qstat", bufs=2) as qstatp:
                    for qi in range(QP):
                        qraw = qrawp.tile([128, CC, HW], F32)
                        deng = nc.sync if qi % 2 == 0 else nc.gpsimd
                        deng.dma_start(
                            qraw[:], x1s[qi].rearrange("(cc p) hw -> p cc hw", p=128)
                        )
                        qsum = qstatp.tile([128, CC], F32, tag="qsum")
                        nc.vector.tensor_reduce(
                            out=qsum[:], in_=qraw[:], axis=mybir.AxisListType.X, op=AL.add
                        )
                        qmneg = qstatp.tile([128, CC], F32, tag="qmneg")
                        nc.scalar.activation(qmneg[:], qsum[:], AF.Copy, scale=-1.0 / HW)
                        g0, g1 = qi * HW, (qi + 1) * HW
                        for i in range(4):
                            a, b = max(g0, PS[i]), min(g1, PS[i + 1])
                            if b <= a:
                                continue
                            for j in range(CC):
                                nc.scalar.activation(
                                    qns[i][:, j, a - PS[i] : b - PS[i]],
                                    qraw[:, j, a - g0 : b - g0],
                                    AF.Identity, bias=qmneg[:, j : j + 1], scale=1.0,
                                )
                                if QMM_MODE == "fp8d":
                                    nc.scalar.activation(
                                        q8ns[i][:, j, a - PS[i] : b - PS[i]],
                                        qraw[:, j, a - g0 : b - g0],
                                        AF.Identity, bias=qmneg[:, j : j + 1],
                                        scale=1.0,
                                    )

                if _rep == 0:
                    for q in range(QP):
                        t0 = (441 * q) // 128
                        s0 = 128 + 128 * t0 - 441 * q
                        nc.sync.dma_start(
                            Wq[:, t0 : t0 + 5, q],
                            cwpad[s0 : s0 + 640].rearrange("(t p) -> p t", p=128),
                        )

                # ---------------- query phase ----------------
                with tc.tile_pool(name="qT", bufs=4) as qTp, \
                     tc.tile_pool(name="q8c", bufs=3) as q8p, \
                     tc.tile_pool(name="r8c", bufs=3) as r8p, \
                     tc.tile_pool(name="prod", bufs=2) as prodp, \
                     tc.tile_pool(name="qcs", bufs=3) as qcsp, \
                     tc.tile_pool(name="qc_ps", bufs=5, space="PSUM") as qc_ps:
                    sttn = 0
                    for t in range(NCH):
                        qnt, tsl = qn_chunk(t)
                        if QMM_MODE == "fp8d":
                            i8 = max(k for k in range(4) if CH[k] <= t)
                            q8nt = q8ns[i8]
                        if QMDT == FP8:
                            tpt = tr_ps.tile([128, C, 2], QMDT, tag="tp")
                            tp = tpt[:, :, 0]
                        else:
                            tpt = tr_ps.tile([128, C], QMDT, tag="tp")
                            tp = tpt[:, :]
                        for j in range(CC):
                            nc.tensor.transpose(
                                tp[:, j * 128 : (j + 1) * 128], qnt[:, j, tsl], identq[:]
                            )
                        qT = qTp.tile([128, C], BF16)
                        nc.scalar.copy(qT[:], tp[:, :])
                        if QMM_MODE == "fp8res":
                            q8c = q8p.tile([128, CC, 128], FP8)
                            nc.gpsimd.tensor_copy(q8c[:], qnt[:, :, tsl])
                            r8c = r8p.tile([128, CC, 128], FP8)
                            nc.gpsimd.tensor_tensor(
                                out=r8c[:], in0=qnt[:, :, tsl], in1=q8c[:],
                                op=AL.subtract,
                            )
                            lhs_list = [q8c, r8c]
                        elif QMM_MODE == "fp8x":
                            q8c = q8p.tile([128, CC, 128], FP8)
                            nc.gpsimd.tensor_copy(q8c[:], qnt[:, :, tsl])
                            lhs_list = None
                        else:
                            lhs_list = None
                        for w in range(WAY):
                            qc = qc_ps.tile([128, C], F32, name=f"qc{t}_{w}", tag="qc")
                            if QMM_MODE == "bf16":
                                for j in range(CC):
                                    nc.tensor.matmul(
                                        qc[:, :],
                                        lhsT=qnt[:, j, tsl],
                                        rhs=cov_w[w][:, j, :],
                                        start=(j == 0), stop=(j == CC - 1),
                                    )
                            elif QMM_MODE == "fp8":
                                for k in range(2):
                                    nc.tensor.matmul(
                                        qc[:, :],
                                        lhsT=qnt[:, 2 * k : 2 * k + 2, tsl],
                                        rhs=cov_w[w][:, 2 * k : 2 * k + 2, :],
                                        start=(k == 0), stop=(k == 1),
                                        perf_mode=DR,
                                    )
                            elif QMM_MODE == "fp8x":
                                for k in range(2):
                                    nc.tensor.matmul(
                                        qc[:, :],
                                        lhsT=q8c[:, 2 * k : 2 * k + 2, :],
                                        rhs=cov_w[w][:, 2 * k : 2 * k + 2, :],
                                        start=(k == 0), stop=(k == 1),
                                        perf_mode=DR,
                                    )
                            elif QMM_MODE == "fp8d":
                                for k in range(2):
                                    nc.tensor.matmul(
                                        qc[:, :],
                                        lhsT=q8nt[:, 2 * k : 2 * k + 2, tsl],
                                        rhs=cov_w[w][:, 2 * k : 2 * k + 2, :],
                                        start=(k == 0), stop=(k == 1),
                                        perf_mode=DR,
                                    )
                            else:   # fp8res
                                for li, lt in enumerate(lhs_list):
                                    for k in range(2):
                                        nc.tensor.matmul(
                                            qc[:, :],
                                            lhsT=lt[:, 2 * k : 2 * k + 2, :],
                                            rhs=cov_w[w][:, 2 * k : 2 * k + 2, :],
                                            start=(li == 0 and k == 0),
                                            stop=(li == 1 and k == 1),
                                            perf_mode=DR,
                                        )
                            sttn += 1
                            if POOL_STT_EVERY and sttn % POOL_STT_EVERY == 0:
                                qcs = qcsp.tile([128, C], BF16)
                                nc.scalar.copy(qcs[:], qc[:])
                                prod = prodp.tile([128, C], BF16)
                                nc.gpsimd.scalar_tensor_tensor(
                                    out=prod[:], in0=qcs[:], scalar=1.0, in1=qT[:],
                                    op0=AL.mult, op1=AL.mult,
                                    accum_out=S_all[:, t, w : w + 1],
                                )
                            elif w < NBOUNCE:
                                # Act bounces qc to SBUF bf16; the DVE STT then
                                # runs in the fast all-SBUF-16bit mode
                                qcs = qcsp.tile([128, C], BF16)
                                nc.scalar.copy(qcs[:], qc[:])
                                prod = prodp.tile([128, C], BF16)
                                nc.vector.scalar_tensor_tensor(
                                    out=prod[:], in0=qcs[:], scalar=1.0, in1=qT[:],
                                    op0=AL.mult, op1=AL.mult,
                                    accum_out=S_all[:, t, w : w + 1],
                                )
                            else:
                                prod = prodp.tile([128, C], FP8 if PROD_FP8 else BF16)
                                nc.vector.scalar_tensor_tensor(
                                    out=prod[:], in0=qc[:], scalar=1.0, in1=qT[:],
                                    op0=AL.mult, op1=AL.mult,
                                    accum_out=S_all[:, t, w : w + 1],
                                )
                # scores = Wq^T @ lrelu(S) accumulated over chunks
                with tc.tile_pool(name="orow", bufs=1) as orowp, \
                     tc.tile_pool(name="sc_ps", bufs=1, space="PSUM") as sc_ps:
                    A = repp.tile([128, NCH, WAY], F32)
                    Sf = S_all[:].rearrange("p t w -> p (t w)")
                    nc.vector.scalar_tensor_tensor(
                        out=A[:].rearrange("p t w -> p (t w)"), in0=Sf, scalar=0.2,
                        in1=Sf, op0=AL.mult, op1=AL.max,
                    )
                    scp = sc_ps.tile([QP, WAY], F32)
                    for t in range(NCH):
                        nc.tensor.matmul(
                            scp[:, :], lhsT=Wq[:, t, :], rhs=A[:, t, :],
                            start=(t == 0), stop=(t == NCH - 1),
                        )
                    orow = orowp.tile([QP, WAY], F32)
                    nc.scalar.copy(orow[:], scp[:])
                    nc.sync.dma_start(scores[:, :], orow[:])
    return nc


def _get_nc(repeat=1):
    key = ("nc", repeat, QMM_MODE, GRAM_FP8, POOL_STT_EVERY, NBOUNCE, PROD_FP8)
    if key not in _cache:
        _install_fixups()
        _cache[key] = _build(repeat)
    return _cache[key]


def kernel(x1, x2, conv_w, conv_b, _trace=False):
    from concourse.bass_utils import run_bass_kernel_spmd

    nc = _get_nc()
    x1 = np.ascontiguousarray(np.asarray(x1, dtype=np.float32)).reshape(Q, C, HW)
    x2 = np.ascontiguousarray(np.asarray(x2, dtype=np.float32)).reshape(WAY * SHOT, C, HW)
    conv_w = np.asarray(conv_w, dtype=np.float32).reshape(HW)
    conv_b = np.asarray(conv_b, dtype=np.float32).reshape(1)

    x1p = np.zeros((NCORES * QP, C, HW), dtype=np.float32)
    x1p[:Q] = x1
    in_maps = [make_in_map(x1p, x2, conv_w, conv_b, c) for c in range(NCORES)]
    res = run_bass_kernel_spmd(nc, in_maps, core_ids=list(range(NCORES)), trace=_trace)
    out = np.concatenate([res.results[c]["scores"] for c in range(NCORES)], axis=0)[:Q]
    out = postprocess(out, conv_b)
    if _trace:
        _cache["last_result"] = res
    return np.ascontiguousarray(out)



# revision 14
# speedup vs baseline: 1.0004x; 1.0004x over previous
"""Trainium2 Bass kernel for nn_Covariance_Metric (5-way 5-shot covariance metric).

Math (see reference):
  cov_w = centered-support covariance (512x512) per way w  [5 ways, 2205 samples each]
  sim[q,w,i] = q_i^T cov_w q_i   for each of 441 spatial positions i of query q
  scores[q,w] = conv_w . leaky_relu(sim[q,w,:]) + conv_b

Strategy: data-parallel over Q across 8 cores (19 queries/core, zero-padded).
Each core computes all 5 covariances (replicated) then its query shard.

Design:
  - Query positions are batched globally: 19*441 = 8379 positions, processed as
    66 chunks of 128 (partition dim = position), so every matmul runs with a
    full 128-row stationary operand (no 441 = 3*128+57 tail waste).
  - QMM_MODE="fp8res": query matmuls run fp8e4 DoubleRow (256-deep contraction,
    0.5 cyc/row) on q8 = fp8(q) AND the requantized residual r8 = fp8(q - q8),
    accumulated in the same PSUM group -> bf16-level accuracy at ~2x bf16 speed.
    The centered query master stays bf16 for the PE transposes that feed the
    DVE diag-product; fp8 tiles are produced per-chunk on the idle GpSimd.
  - Gram (covariance) phase: x2 is converted to fp8 on GpSimd, PE-transposed
    (1 cyc/row), and accumulated with fp8 DR matmuls; channel sums come from a
    ones-vector DR matmul on PE; mean correction is a rank-1 fp32r matmul.
    cov is stored as COV_SCALE*cov in fp8 (off-diagonals ~0.02 would fall into
    e4m3 denormals otherwise); LeakyReLU is positively homogeneous so the scale
    passes through to the scores and is undone on host in postprocess().
  - sim diag-product via scalar_tensor_tensor free-dim accumulate; most run on
    DVE straight from PSUM, every POOL_STT_EVERY-th way-chunk is bounced
    PSUM->SBUF-bf16 by the Scalar engine and reduced on GpSimd to balance load.
  - scores: LeakyReLU on DVE, then 66 accumulating mask-matmuls with conv_w
    scattered into a [128 pos, 66 chunk, 19 query] weight tensor; conv_b is
    added on host.
"""
import json
import numpy as np
from contextlib import ExitStack

import concourse.bass as bass
import concourse.tile as tile
from concourse import mybir
from concourse.masks import make_identity

# ---------------------------------------------------------------------------
# Workaround for this container's walrus build: it supports only ONE sync-wait
# command per instruction, while Tile attaches several. Rewrite the BIR JSON to
# hoist extra waits onto inserted same-engine NoOps (the NX sequencer processes
# them in order, so the gating is equivalent).
# ---------------------------------------------------------------------------

def _split_sync_waits_json(m: dict) -> int:
    nsplit = 0
    for fn in m["functions"]:
        for bb in fn["blocks"]:
            out = []
            for ins in bb["instructions"]:
                si = ins.get("sync_info")
                if si:
                    w = si.get("on_wait") or []
                    if len(w) > 1:
                        for c in w[:-1]:
                            nsplit += 1
                            out.append({
                                "debug": ins.get("debug", 0),
                                "engine": ins["engine"],
                                "ins": [], "outs": [],
                                "name": f"{ins['name']}-ws{nsplit}",
                                "opcode": "NoOp",
                                "sync_info": {"on_wait": [c], "on_update": []},
                            })
                        si["on_wait"] = [w[-1]]
                out.append(ins)
            bb["instructions"] = out
    return nsplit


_fixups_installed = False


def _install_fixups():
    global _fixups_installed
    if _fixups_installed:
        return
    _fixups_installed = True
    import concourse.bass_utils as bu
    import concourse.bass2jax as b2j

    orig = bu.compile_bir_kernel

    def compile_bir_kernel_patched(bir_json, tmpdir, neff_name="file.neff"):
        m = json.loads(bir_json)
        _split_sync_waits_json(m)
        return orig(json.dumps(m).encode(), tmpdir, neff_name)

    bu.compile_bir_kernel = compile_bir_kernel_patched
    b2j.compile_bir_kernel = compile_bir_kernel_patched


# ---------------------------------------------------------------------------
# Problem constants (hardcoded per contract)
# ---------------------------------------------------------------------------
Q, C, HW = 150, 512, 441
WAY, SHOT = 5, 5
NCORES = 8
QP = 19             # queries per core (8*19 = 152 >= 150, zero-padded)
CC = C // 128       # 4 c-chunks
NS = SHOT * HW      # 2205 samples per way
P = QP * HW         # 8379 positions per core
NCH = (P + 127) // 128   # 66 position chunks
PCH = NCH * 128          # 8448 padded positions
HWP = 512                # per-shot hw dim padded to 4 full 128-chunks
F32 = mybir.dt.float32
F32R = mybir.dt.float32r
BF16 = mybir.dt.bfloat16
FP8 = mybir.dt.float8e4
DR = mybir.MatmulPerfMode.DoubleRow

# tunables
QMM_MODE = "fp8x"     # "bf16" | "fp8" | "fp8res" | "fp8x" | "fp8d"
                      # fp8x: bf16 master for transposes/qT (exact diag
                      # multiplicand), per-chunk GpSimd fp8 copy for the
                      # DoubleRow matmuls (halves PE query time; cov fp8)
GRAM_FP8 = True       # gram matmuls in fp8 DoubleRow (else fp32r, no x2 pad)
POOL_STT_EVERY = 0    # 0 = all sim-reductions on DVE (TensorScalarPtr is
                      # not a legal GpSimd opcode on hw); k>0 kept for expts
                      # way-chunk bounced via ACT copy + GpSimd reduce
NBOUNCE = 0           # ways per chunk whose sim-reduction is bounced
                      # PSUM->SBUF-bf16 by Act, then STT on DVE in fast mode
                      # (measured: regression, Act relay stalls the pipeline)
PROD_FP8 = False      # write the discarded STT product in fp8 (halves the
                      # DVE SBUF write footprint per sim-reduction)
GP_SPLIT = 0          # ways per chunk whose sim-reduction runs as GpSimd
                      # tensor_tensor mult (PSUM x qT -> SBUF bf16) + DVE
                      # tensor_reduce in 16-bit fast mode
COV_SCALE = 64.0      # see module docstring

_cache = {}
HOST_BIAS = True      # conv_b (and 1/COV_SCALE) applied on host in kernel()


def postprocess(scores_dev, conv_b):
    return scores_dev * np.float32(1.0 / COV_SCALE) + np.float32(conv_b[0])


def make_in_map(x1p, x2, cw, cb, c):
    return {
        "x1s": np.ascontiguousarray(x1p[c * QP : (c + 1) * QP]),
        "x2": x2, "cw": cw,
    }


def _wq_segments():
    """(chunk t, dst partition range [a,b), query q, conv_w offset i0)."""
    segs = []
    for t in range(NCH):
        g0, g1 = 128 * t, 128 * t + 128
        q_lo, q_hi = g0 // HW, min((g1 - 1) // HW, QP - 1)
        for q in range(q_lo, q_hi + 1):
            s0, s1 = max(g0, q * HW), min(g1, (q + 1) * HW, P)
            if s1 > s0:
                segs.append((t, s0 - g0, s1 - g0, q, s0 - q * HW))
    return segs


def _build(repeat=1):
    nc = bass.Bass(trn_type="TRN2")
    x1s = nc.dram_tensor("x1s", [QP, C, HW], F32, kind="ExternalInput")
    x2 = nc.dram_tensor("x2", [WAY * SHOT, C, HW], F32, kind="ExternalInput")
    cw = nc.dram_tensor("cw", [HW], F32, kind="ExternalInput")
    scores = nc.dram_tensor("scores", [QP, WAY], F32, kind="ExternalOutput")

    AL = mybir.AluOpType
    AF = mybir.ActivationFunctionType
    QMDT = FP8 if QMM_MODE == "fp8" else BF16     # dtype of the query master
    GDT = FP8 if GRAM_FP8 else F32R

    with tile.TileContext(nc) as tc, ExitStack() as ctx:
        consts = ctx.enter_context(tc.tile_pool(name="consts", bufs=1))
        tr_ps = ctx.enter_context(tc.tile_pool(name="tr_ps", bufs=3, space="PSUM"))

        identq = consts.tile([128, 128], QMDT)
        make_identity(nc, identq[:])
        identg = consts.tile([128, 128], FP8 if GRAM_FP8 else F32)
        make_identity(nc, identg[:])
        # k-pair dim stride must be a multiple of 16 bytes for DoubleRow LDW
        ones_g = consts.tile([128, 2, 16], GDT)
        nc.gpsimd.memset(ones_g[:], 1.0)
        # conv_w scattered by (position-in-chunk, chunk, query):
        # Wq[p, t, q] = cw[128t + p - 441q] if 0 <= idx < 441 else 0.
        # Build a zero-padded DRAM copy cwpad[k] = cw[k-128] so each query is
        # ONE affine 2D DMA (the pad supplies the zero corners of the band).
        Wq = consts.tile([128, NCH + 1, QP], F32)
        nc.gpsimd.memset(Wq[:], 0.0)
        cwpad = nc.dram_tensor("cwpad", [768], F32, kind="Internal")
        zrow = consts.tile([1, 256], F32)
        nc.vector.memset(zrow[:], 0.0)
        nc.sync.dma_start(cwpad[0:128][None, :], zrow[0:1, 0:128])
        nc.sync.dma_start(cwpad[569:768][None, :], zrow[0:1, 0:199])
        nc.sync.dma_start(cwpad[128:569][None, :], cw[:][None, :])

        for _rep in range(repeat):
            with tc.tile_pool(name="rep", bufs=1) as repp:
                covdt = FP8 if QMM_MODE != "bf16" else BF16
                cov_w = [repp.tile([128, CC, C], covdt, name=f"cov{w}")
                         for w in range(WAY)]
                # centered queries, c-major, split at chunk boundaries so
                # early chunks only wait for their own queries' centering
                # (tile deps are whole-tile, not range-based)
                CH = [0, 17, 34, 50, 66]
                PS = [128 * c for c in CH]
                qns = [repp.tile([128, CC, PS[i + 1] - PS[i]], QMDT, name=f"qn{i}")
                       for i in range(4)]
                S_all = repp.tile([128, NCH, WAY], F32)
                nc.gpsimd.memset(qns[3][:, :, P - PS[3] :], 0.0)
                if QMM_MODE == "fp8d":
                    q8ns = [repp.tile([128, CC, PS[i + 1] - PS[i]], FP8,
                                      name=f"q8n{i}") for i in range(4)]
                    nc.gpsimd.memset(q8ns[3][:, :, P - PS[3] :], 0.0)

                def qn_chunk(t):
                    i = max(k for k in range(4) if CH[k] <= t)
                    return qns[i], slice(128 * t - PS[i], 128 * t - PS[i] + 128)

                # ---------------- covariance phase ----------------
                with tc.tile_pool(name="x2s", bufs=2) as x2pool, \
                     tc.tile_pool(name="x2f", bufs=2) as x2fp, \
                     tc.tile_pool(name="fT", bufs=3) as fTp, \
                     tc.tile_pool(name="murow", bufs=2) as murp, \
                     tc.tile_pool(name="g_ps", bufs=4, space="PSUM") as g_ps, \
                     tc.tile_pool(name="mu_ps", bufs=1, space="PSUM") as mu_ps:
                    for w in range(WAY):
                        g = [g_ps.tile([128, C], F32, name=f"g{w}_{j}", tag="g")
                             for j in range(CC)]
                        mrow_ps = mu_ps.tile([1, C], F32)
                        first = True
                        for s in range(SHOT):
                            x2s = x2pool.tile([128, CC, HW], F32)
                            nc.sync.dma_start(
                                x2s[:],
                                x2[w * SHOT + s].rearrange("(cc p) hw -> p cc hw", p=128),
                            )
                            # fp8 copy with zero-padded hw 441->512 (GpSimd)
                            x2f = x2fp.tile([128, CC, HWP], GDT)
                            nc.gpsimd.memset(x2f[:, :, HW:HWP], 0.0)
                            nc.gpsimd.tensor_copy(x2f[:, :, :HW], x2s[:])
                            for pair in range(2):   # hw chunks (2p, 2p+1)
                                fT = fTp.tile([128, 2, C], GDT)
                                for slot in range(2):
                                    h = 2 * pair + slot
                                    if GRAM_FP8:
                                        tpt = tr_ps.tile([128, C, 2], GDT, tag="tp")
                                        tp = tpt[:, :, 0]   # fp8 transpose needs
                                        # output element step of 2 (walrus)
                                    else:
                                        tpt = tr_ps.tile([128, C], GDT, tag="tp")
                                        tp = tpt[:, :]
                                    for j in range(CC):
                                        nc.tensor.transpose(
                                            tp[:, j * 128 : (j + 1) * 128],
                                            x2f[:, j, h * 128 : (h + 1) * 128],
                                            identg[:],
                                        )
                                    if (pair + slot) % 2 == 0:
                                        nc.scalar.copy(fT[:, slot, :], tp[:, :])
                                    else:
                                        nc.vector.tensor_copy(fT[:, slot, :], tp[:, :])
                                for j in range(CC):
                                    nc.tensor.matmul(
                                        g[j][:, :],
                                        lhsT=fT[:, :, j * 128 : (j + 1) * 128],
                                        rhs=fT[:, :, :],
                                        start=first, stop=False,
                                        perf_mode=DR,
                                    )
                                nc.tensor.matmul(
                                    mrow_ps[0:1, :],
                                    lhsT=ones_g[:, :, 0:1],
                                    rhs=fT[:, :, :],
                                    start=first, stop=(s == SHOT - 1 and pair == 1),
                                    perf_mode=DR,
                                )
                                first = False
                        murow = murp.tile([1, C], F32R)    # mu = sums / NS
                        nc.scalar.activation(murow[:], mrow_ps[:], AF.Copy, scale=1.0 / NS)
                        mursc = murp.tile([1, C], F32R)    # -NS*mu = -sums
                        nc.scalar.activation(mursc[:], mrow_ps[:], AF.Copy, scale=-1.0)
                        for j in range(CC):
                            nc.tensor.matmul(
                                g[j][:, :],
                                lhsT=mursc[0:1, j * 128 : (j + 1) * 128],
                                rhs=murow[0:1, :],
                                start=False, stop=True,
                            )
                        for j in range(CC):
                            nc.scalar.activation(
                                cov_w[w][:, j, :], g[j][:, :], AF.Copy,
                                scale=COV_SCALE / (NS - 1),
                            )

                # ---------------- query load + centering ----------------
                with tc.tile_pool(name="qraw", bufs=2) as qrawp, \
                     tc.tile_pool(name="qstat", bufs=2) as qstatp:
                    for qi in range(QP):
                        qraw = qrawp.tile([128, CC, HW], F32)
                        deng = nc.sync if qi % 2 == 0 else nc.gpsimd
                        deng.dma_start(
                            qraw[:], x1s[qi].rearrange("(cc p) hw -> p cc hw", p=128)
                        )
                        qsum = qstatp.tile([128, CC], F32, tag="qsum")
                        nc.vector.tensor_reduce(
                            out=qsum[:], in_=qraw[:], axis=mybir.AxisListType.X, op=AL.add
                        )
                        qmneg = qstatp.tile([128, CC], F32, tag="qmneg")
                        nc.scalar.activation(qmneg[:], qsum[:], AF.Copy, scale=-1.0 / HW)
                        g0, g1 = qi * HW, (qi + 1) * HW
                        for i in range(4):
                            a, b = max(g0, PS[i]), min(g1, PS[i + 1])
                            if b <= a:
                                continue
                            for j in range(CC):
                                nc.scalar.activation(
                                    qns[i][:, j, a - PS[i] : b - PS[i]],
                                    qraw[:, j, a - g0 : b - g0],
                                    AF.Identity, bias=qmneg[:, j : j + 1], scale=1.0,
                                )
                                if QMM_MODE == "fp8d":
                                    nc.scalar.activation(
                                        q8ns[i][:, j, a - PS[i] : b - PS[i]],
                                        qraw[:, j, a - g0 : b - g0],
                                        AF.Identity, bias=qmneg[:, j : j + 1],
                                        scale=1.0,
                                    )

                if _rep == 0:
                    for q in range(QP):
                        t0 = (441 * q) // 128
                        s0 = 128 + 128 * t0 - 441 * q
                        nc.sync.dma_start(
                            Wq[:, t0 : t0 + 5, q],
                            cwpad[s0 : s0 + 640].rearrange("(t p) -> p t", p=128),
                        )

                # ---------------- query phase ----------------
                with tc.tile_pool(name="qT", bufs=4) as qTp, \
                     tc.tile_pool(name="q8c", bufs=3) as q8p, \
                     tc.tile_pool(name="r8c", bufs=3) as r8p, \
                     tc.tile_pool(name="prod", bufs=2) as prodp, \
                     tc.tile_pool(name="qcs", bufs=3) as qcsp, \
                     tc.tile_pool(name="qc_ps", bufs=5, space="PSUM") as qc_ps:
                    sttn = 0
                    for t in range(NCH):
                        qnt, tsl = qn_chunk(t)
                        if QMM_MODE == "fp8d":
                            i8 = max(k for k in range(4) if CH[k] <= t)
                            q8nt = q8ns[i8]
                        if QMDT == FP8:
                            tpt = tr_ps.tile([128, C, 2], QMDT, tag="tp")
                            tp = tpt[:, :, 0]
                        else:
                            tpt = tr_ps.tile([128, C], QMDT, tag="tp")
                            tp = tpt[:, :]
                        for j in range(CC):
                            nc.tensor.transpose(
                                tp[:, j * 128 : (j + 1) * 128], qnt[:, j, tsl], identq[:]
                            )
                        qT = qTp.tile([128, C], BF16)
                        nc.scalar.copy(qT[:], tp[:, :])
                        if QMM_MODE == "fp8res":
                            q8c = q8p.tile([128, CC, 128], FP8)
                            nc.gpsimd.tensor_copy(q8c[:], qnt[:, :, tsl])
                            r8c = r8p.tile([128, CC, 128], FP8)
                            nc.gpsimd.tensor_tensor(
                                out=r8c[:], in0=qnt[:, :, tsl], in1=q8c[:],
                                op=AL.subtract,
                            )
                            lhs_list = [q8c, r8c]
                        elif QMM_MODE == "fp8x":
                            q8c = q8p.tile([128, CC, 128], FP8)
                            nc.gpsimd.tensor_copy(q8c[:], qnt[:, :, tsl])
                            lhs_list = None
                        else:
                            lhs_list = None
                        for w in range(WAY):
                            qc = qc_ps.tile([128, C], F32, name=f"qc{t}_{w}", tag="qc")
                            if QMM_MODE == "bf16":
                                for j in range(CC):
                                    nc.tensor.matmul(
                                        qc[:, :],
                                        lhsT=qnt[:, j, tsl],
                                        rhs=cov_w[w][:, j, :],
                                        start=(j == 0), stop=(j == CC - 1),
                                    )
                            elif QMM_MODE == "fp8":
                                for k in range(2):
                                    nc.tensor.matmul(
                                        qc[:, :],
                                        lhsT=qnt[:, 2 * k : 2 * k + 2, tsl],
                                        rhs=cov_w[w][:, 2 * k : 2 * k + 2, :],
                                        start=(k == 0), stop=(k == 1),
                                        perf_mode=DR,
                                    )
                            elif QMM_MODE == "fp8x":
                                for k in range(2):
                                    nc.tensor.matmul(
                                        qc[:, :],
                                        lhsT=q8c[:, 2 * k : 2 * k + 2, :],
                                        rhs=cov_w[w][:, 2 * k : 2 * k + 2, :],
                                        start=(k == 0), stop=(k == 1),
                                        perf_mode=DR,
                                    )
                            elif QMM_MODE == "fp8d":
                                for k in range(2):
                                    nc.tensor.matmul(
                                        qc[:, :],
                                        lhsT=q8nt[:, 2 * k : 2 * k + 2, tsl],
                                        rhs=cov_w[w][:, 2 * k : 2 * k + 2, :],
                                        start=(k == 0), stop=(k == 1),
                                        perf_mode=DR,
                                    )
                            else:   # fp8res
                                for li, lt in enumerate(lhs_list):
                                    for k in range(2):
                                        nc.tensor.matmul(
                                            qc[:, :],
                                            lhsT=lt[:, 2 * k : 2 * k + 2, :],
                                            rhs=cov_w[w][:, 2 * k : 2 * k + 2, :],
                                            start=(li == 0 and k == 0),
                                            stop=(li == 1 and k == 1),
                                            perf_mode=DR,
                                        )
                            sttn += 1
                            if POOL_STT_EVERY and sttn % POOL_STT_EVERY == 0:
                                qcs = qcsp.tile([128, C], BF16)
                                nc.scalar.copy(qcs[:], qc[:])
                                prod = prodp.tile([128, C], BF16)
                                nc.gpsimd.scalar_tensor_tensor(
                                    out=prod[:], in0=qcs[:], scalar=1.0, in1=qT[:],
                                    op0=AL.mult, op1=AL.mult,
                                    accum_out=S_all[:, t, w : w + 1],
                                )
                            elif w < GP_SPLIT:
                                prodg = prodp.tile([128, C], BF16)
                                nc.gpsimd.tensor_tensor(
                                    out=prodg[:], in0=qc[:], in1=qT[:],
                                    op=AL.mult,
                                )
                                nc.vector.tensor_reduce(
                                    out=S_all[:, t, w : w + 1],
                                    in_=prodg[:],
                                    axis=mybir.AxisListType.X, op=AL.add,
                                )
                            elif w < GP_SPLIT + NBOUNCE:
                                # Act bounces qc to SBUF bf16; the DVE STT then
                                # runs in the fast all-SBUF-16bit mode
                                qcs = qcsp.tile([128, C], BF16)
                                nc.scalar.copy(qcs[:], qc[:])
                                prod = prodp.tile([128, C], BF16)
                                nc.vector.scalar_tensor_tensor(
                                    out=prod[:], in0=qcs[:], scalar=1.0, in1=qT[:],
                                    op0=AL.mult, op1=AL.mult,
                                    accum_out=S_all[:, t, w : w + 1],
                                )
                            else:
                                prod = prodp.tile([128, C], FP8 if PROD_FP8 else BF16)
                                nc.vector.scalar_tensor_tensor(
                                    out=prod[:], in0=qc[:], scalar=1.0, in1=qT[:],
                                    op0=AL.mult, op1=AL.mult,
                                    accum_out=S_all[:, t, w : w + 1],
                                )
                # scores = Wq^T @ lrelu(S) accumulated over chunks
                with tc.tile_pool(name="orow", bufs=1) as orowp, \
                     tc.tile_pool(name="sc_ps", bufs=1, space="PSUM") as sc_ps:
                    A = repp.tile([128, NCH, WAY], F32)
                    Sf = S_all[:].rearrange("p t w -> p (t w)")
                    nc.vector.scalar_tensor_tensor(
                        out=A[:].rearrange("p t w -> p (t w)"), in0=Sf, scalar=0.2,
                        in1=Sf, op0=AL.mult, op1=AL.max,
                    )
                    scp = sc_ps.tile([QP, WAY], F32)
                    for t in range(NCH):
                        nc.tensor.matmul(
                            scp[:, :], lhsT=Wq[:, t, :], rhs=A[:, t, :],
                            start=(t == 0), stop=(t == NCH - 1),
                        )
                    orow = orowp.tile([QP, WAY], F32)
                    nc.scalar.copy(orow[:], scp[:])
                    nc.sync.dma_start(scores[:, :], orow[:])
    return nc


def _get_nc(repeat=1):
    key = ("nc", repeat, QMM_MODE, GRAM_FP8, POOL_STT_EVERY, NBOUNCE, PROD_FP8, GP_SPLIT)
    if key not in _cache:
        _install_fixups()
        _cache[key] = _build(repeat)
    return _cache[key]


def kernel(x1, x2, conv_w, conv_b, _trace=False):
    from concourse.bass_utils import run_bass_kernel_spmd

    nc = _get_nc()
    x1 = np.ascontiguousarray(np.asarray(x1, dtype=np.float32)).reshape(Q, C, HW)
    x2 = np.ascontiguousarray(np.asarray(x2, dtype=np.float32)).reshape(WAY * SHOT, C, HW)
    conv_w = np.asarray(conv_w, dtype=np.float32).reshape(HW)
    conv_b = np.asarray(conv_b, dtype=np.float32).reshape(1)

    x1p = np.zeros((NCORES * QP, C, HW), dtype=np.float32)
    x1p[:Q] = x1
    in_maps = [make_in_map(x1p, x2, conv_w, conv_b, c) for c in range(NCORES)]
    res = run_bass_kernel_spmd(nc, in_maps, core_ids=list(range(NCORES)), trace=_trace)
    out = np.concatenate([res.results[c]["scores"] for c in range(NCORES)], axis=0)[:Q]
    out = postprocess(out, conv_b)
    if _trace:
        _cache["last_result"] = res
    return np.ascontiguousarray(out)

